# revision 32
# baseline (speedup 1.0000x reference)
"""KoLeo loss kernel for 8 Trainium2 NeuronCores — symmetric (half-matrix)
variant, v2.

Reference computation (B=16384, D=1024):
    xn  = x / max(||x||_2, 1e-12)          # row L2-normalize
    sim = xn @ xn.T                        # B x B cosine similarity
    max_sim[i] = max_{j != i} sim[i, j]    # nearest neighbor (excl. self)
    out = -mean(log(sqrt(2 - 2*max_sim + 1e-8)))

sim is symmetric, so only the upper triangle of 512x512 super-blocks is
computed. Work distribution ("pencil window", SPMD-uniform): global
super-row G is owned by core c = G % 8. Each core holds its 4 owned
super-rows resident and computes super-blocks (I, I+w mod 32) for
w = 0..16 (a in {0,1}) or w = 0..15 (a in {2,3}); every unordered pair
of super-blocks is covered exactly once across the fleet (528 total).

Measured (8-core SPMD, core 0 profile): ~249us on a 2.4 GHz run,
~296us when the chip's P0 power limit drops the PE to ~2.0 GHz
(uncontrollable, ~25% of runs). Breakdown of a warm run: ~7us engine
preamble, first matmul at ~9us, 225.8us matmul stream (99% dense,
216 ns per 512-wide fp8 DoubleRow matmul = the 2.4 GHz streaming
roofline), ~4us epilogue tail, ~10us framework teardown.

Changes vs v1 (which ran PE and DVE both ~90% busy at ~3.85us/block,
258-308us total depending on the chip power state):
  - Diagonal (w=0) super-blocks only compute their upper-triangle
    128-row chunks (rows chunk r x cols 128r..512): 5120 instead of
    8192 PE cycles per diag block. A ragged column-max tile (cols
    128..512, max over the computed rows, self-sims masked to -130+64)
    is shipped so the skipped lower-triangle pairs stay covered.
  - DVE row-max restructure: instead of a full 2.2us tensor_reduce per
    block, a running row-max accumulator racc[a] takes one f16 2x-mode
    tensor_max per block (~1.1us); the expensive reduce happens once
    per window. DVE drops from ~3.85us/block (co-bottleneck with PE)
    to ~2.1us/block. (tensor_tensor_reduce would fuse this further but
    faults the exec unit on this runtime — verified by HW probe.)
  - rhs blocks at local index 8k (w = 8, 16) are the resident lhs
    tiles — 6 fewer 512KB HBM reads.
  - All DMA issues move off the Scalar queue (ACT was doing 44us of
    DMA_DIRECT2D issue on top of its 156us of PSUM-drain copies):
    inputs on Sync, outputs on GpSimd (otherwise idle).
  - ~24 tiny f16 warm-up matmuls on the eye tile run during the input
    DMA window so the PE's HAM clock-gate (cold 1.2 GHz for the first
    ~3.4us of activity) warms up on garbage instead of real work.

Per-block engine budget (warm, 2.4 GHz PE): PE 3.41us (16 fp8 DoubleRow
matmuls, N=512, zero inter-MM bubble measured), ACT 2.36us (PSUM->f16
SBUF stage copy, sole bulk PSUM consumer), DVE ~2.1us, leaving PE the
sole bottleneck at ~220us/core + start/tail.

Host: pre-normalizes rows (f64), scales by 8 and casts to fp8e4m3,
pre-rotates/retiles per core; post-merges row/column maxima across
cores and applies the scalar log epilogue in f64.
"""

import sys

if "/opt/trn_rl_repo" not in sys.path:
    sys.path.insert(0, "/opt/trn_rl_repo")

import numpy as np
import ml_dtypes

import concourse.bass as bass  # noqa: F401  (import keeps bass registered)
import concourse.mybir as mybir
import concourse.tile as tile
from concourse import bacc
from concourse.bass_utils import run_bass_kernel_spmd

P = 128          # SBUF partitions
NBLK = 512       # super-block side (= one PSUM bank of f32 per 128 rows)
EPS = 1e-8

B = 16384        # rows of x
D = 1024         # feature dim
N_CORES = 8
NSB = B // NBLK  # 32 super-blocks per matrix side
KCH = D // P     # 8 contraction chunks of 128
KSTEP = 2        # fp8 DoubleRow: K chunks of 256 per matmul
NA = 4           # owned super-rows per core (global stride 8)
FP8_SCALE = 8.0
EYE_VAL = -130.0  # added at self-sim positions (value 64) before maxes
N_WARMUP = 18    # f16 eye matmuls to heat the PE HAM clock-gate


def _windows():
    """Program-order (a, w) list. w=0 is the diagonal super-block.

    a in {0,1} get w up to 16, a in {2,3} up to 15: the distance-16 pairs
    {i, i+16} are covered once by the a0/a1 windows (i = c + 8*a0), so the
    a2/a3 windows stop at 15. Total 66 super-blocks per core; the union of
    (owned I, I+w) over all cores covers every unordered block pair once.
    """
    out = []
    for a in range(NA):
        wmax = 16 if a < 2 else 15
        for w in range(wmax + 1):
            out.append((a, w))
    return out


N_SLOTS = len(_windows())  # 66: every block ships a colmax tile now
DIAG_W = 384               # diag colmax covers block cols 128..512


def build_nc():
    """Build the per-core SPMD Bass program.

    Inputs :  xt     [NSB*P, KCH*NBLK] fp8e4m3 — normalized, scaled x.T,
              retiled as [J, p, k, j] and column-rotated by 512*c so owned
              super-rows sit at local block 8a.
              eyef16 [P, P] f16 — the constant EYE_VAL * I
    Outputs:  rowmax [P, 16]          f32 — [p, 4a+r] = row-max over the
              computed window for local row 4096a + 128r + p
              colmax [N_SLOTS*P, NBLK] f16 — per super-block (program
              order), the r-chunk-folded [128, 512] column-max tile (for
              diagonal blocks only cols 0:384 are valid, covering block
              cols 128..512); the host reduces the 128 partitions.
    """
    f32 = mybir.dt.float32
    f16 = mybir.dt.float16
    fp8 = mybir.dt.float8e4
    ngrp = KCH // KSTEP

    nc = bacc.Bacc("TRN2", target_bir_lowering=False, debug=False,
                   num_devices=N_CORES)
    xt = nc.dram_tensor("xt", [NSB * P, KCH * NBLK], fp8,
                        kind="ExternalInput")
    eyed = nc.dram_tensor("eyef16", [P, P], f16, kind="ExternalInput")
    rowmax_d = nc.dram_tensor("rowmax", [P, NA * 4], f32,
                              kind="ExternalOutput")
    colmax_d = nc.dram_tensor("colmax", [N_SLOTS * P, NBLK], f16,
                              kind="ExternalOutput")
    xt_ap = xt[:]
    colmax_ap = colmax_d[:]

    with tile.TileContext(nc) as tc:
        with (
            tc.tile_pool(name="lhs", bufs=1) as lhs_pool,
            tc.tile_pool(name="rhs", bufs=3) as rhs_pool,
            tc.tile_pool(name="psum", bufs=4, space="PSUM") as psum_pool,
            tc.tile_pool(name="stage", bufs=4) as stage_pool,
            tc.tile_pool(name="stats", bufs=1) as stats_pool,
        ):
            # --- input DMAs, all on the Sync queue. Order matters: lhs0
            # first (the first super-block needs only it), then the eye
            # (not consumed until the first diagonal epilogue); lhs1..3
            # are issued from inside the loop so early rhs prefetches
            # aren't stuck behind them.
            eye = stats_pool.tile([P, P], f16, name="eye")

            lhs_tiles = [
                lhs_pool.tile([P, KCH, NBLK], fp8, name=f"lhs{a}",
                              tag=f"lhs{a}")
                for a in range(NA)
            ]

            def load_lhs(a):
                rows = slice(8 * a * P, (8 * a + 1) * P)
                nc.sync.dma_start(lhs_tiles[a][:], xt_ap[rows, :])

            load_lhs(0)
            nc.sync.dma_start(eye[:], eyed[:])

            racc = [
                stats_pool.tile([P, 4, NBLK], f16, name=f"racc{a}",
                                tag=f"racc{a}")
                for a in range(NA)
            ]
            rowmax_sb = stats_pool.tile([P, NA * 4], f32, name="rowmax_sb")
            rowtmp = stats_pool.tile([P, 4], f32, name="rowtmp")
            # scratch for the last block's j-fold tree
            stj = stats_pool.tile([P, 4, NBLK // 2], f16, name="stj")
            stj2 = stats_pool.tile([P, 4, NBLK // 4], f16, name="stj2")

            # --- PE warm-up: tiny matmuls on a never-written SBUF
            # scratch tile (garbage data, zero input dependencies — they
            # start the moment the Tensor queue is up, before any DMA
            # lands) fill the HAM activity window while lhs0 streams in.
            # They write into the first block's pre-allocated psum half;
            # its first real matmul's start=True clears has_written. ---
            wsc = stats_pool.tile([P, P], f16, name="wsc")
            nc.vector.memset(wsc[:], 0.25)
            first_ps = psum_pool.tile([P, 2, NBLK], f32, name="ps",
                                      tag="ps")
            for _ in range(N_WARMUP):
                nc.tensor.matmul(first_ps[:, 0, 0:P], wsc[:], wsc[:],
                                 start=True, stop=True)

            def row_reduce(dst_ap, src_tile):
                """dst[:, 0:4] = per-chunk row max of src [P, 4, NBLK]."""
                nc.vector.reduce_max(
                    out=dst_ap,
                    in_=src_tile[:],
                    axis=mybir.AxisListType.X,
                    op=mybir.AluOpType.max,
                )

            sb_idx = 0
            for a, w in _windows():
                L = (8 * a + w) % NSB
                wmax = 16 if a < 2 else 15
                if L % 8 == 0:
                    rt = lhs_tiles[L // 8]  # resident (w = 0, 8, 16)
                else:
                    rt = rhs_pool.tile([P, KCH, NBLK], fp8, name="rt",
                                       tag="rt")
                    nc.sync.dma_start(rt[:], xt_ap[L * P:(L + 1) * P, :])
                if a == 0 and 1 <= w <= 3:
                    load_lhs(w)  # deferred so early rhs loads go first

                # two 2-bank psum halves per block: ACT staging starts
                # after 8 matmuls instead of 16 and the PE can run two
                # blocks ahead of the epilogue (4 pool bufs = 8 banks)
                if first_ps is not None:
                    psh0, first_ps = first_ps, None
                else:
                    psh0 = psum_pool.tile([P, 2, NBLK], f32, name="ps",
                                          tag="ps")
                st = stage_pool.tile([P, 4, NBLK], f16, name="st", tag="st")
                if w == 0:
                    # diagonal: chunk r covers block cols 128r..512 only
                    psh1 = None
                    for r in range(4):
                        if r == 2:
                            psh1 = psum_pool.tile([P, 2, NBLK], f32,
                                                  name="ps", tag="ps")
                        ph = psh0 if r < 2 else psh1
                        nw = NBLK - r * P
                        for g in range(ngrp):
                            ks = slice(KSTEP * g, KSTEP * (g + 1))
                            nc.tensor.matmul(
                                ph[:, r % 2, 0:nw],
                                lhs_tiles[a][:, ks, r * P:(r + 1) * P],
                                lhs_tiles[a][:, ks, r * P:NBLK],
                                start=(g == 0),
                                stop=(g == ngrp - 1),
                                perf_mode=mybir.MatmulPerfMode.DoubleRow,
                            )
                        if r == 1:  # stage half 0 while half 1 computes
                            nc.scalar.copy(st[:, 0, :], psh0[:, 0, :])
                            nc.scalar.copy(st[:, 1, 0:NBLK - P],
                                           psh0[:, 1, 0:NBLK - P])
                    nc.scalar.copy(st[:, 2, 0:NBLK - 2 * P],
                                   psh1[:, 0, 0:NBLK - 2 * P])
                    nc.scalar.copy(st[:, 3, 0:NBLK - 3 * P],
                                   psh1[:, 1, 0:NBLK - 3 * P])
                    # self-sim sits at st[p, r, p]: mask it below any
                    # real similarity before any max consumes it
                    for r in range(4):
                        sl = st[:, r, 0:P]
                        nc.vector.tensor_add(out=sl, in0=sl, in1=eye[:])
                    # init racc: ragged copy + -inf tails
                    nc.vector.tensor_copy(racc[a][:, 0, :], st[:, 0, :])
                    for r in range(1, 4):
                        nw = NBLK - r * P
                        nc.vector.tensor_copy(racc[a][:, r, 0:nw],
                                              st[:, r, 0:nw])
                        nc.vector.memset(racc[a][:, r, nw:NBLK], -60000.0)
                    # ragged column-max over the computed rows, block
                    # cols 128..512 (col offset 128r+j in chunk r maps to
                    # local j after the per-chunk 128-col shift), folded
                    # in place into st chunk 0 (racc copies ran already)
                    nc.vector.tensor_max(out=st[:, 0, P:NBLK],
                                         in0=st[:, 0, P:NBLK],
                                         in1=st[:, 1, 0:DIAG_W])
                    nc.vector.tensor_max(out=st[:, 0, 2 * P:NBLK],
                                         in0=st[:, 0, 2 * P:NBLK],
                                         in1=st[:, 2, 0:NBLK - 2 * P])
                    nc.vector.tensor_max(out=st[:, 0, 3 * P:NBLK],
                                         in0=st[:, 0, 3 * P:NBLK],
                                         in1=st[:, 3, 0:P])
                    nc.sync.dma_start(
                        colmax_ap[sb_idx * P:(sb_idx + 1) * P, 0:DIAG_W],
                        st[:, 0, P:NBLK])
                else:
                    last = (a == NA - 1 and w == wmax)
                    psh1 = None
                    for r in range(4):
                        if r == 2:
                            psh1 = psum_pool.tile([P, 2, NBLK], f32,
                                                  name="ps", tag="ps")
                        ph = psh0 if r < 2 else psh1
                        for g in range(ngrp):
                            ks = slice(KSTEP * g, KSTEP * (g + 1))
                            nc.tensor.matmul(
                                ph[:, r % 2, :],
                                lhs_tiles[a][:, ks, r * P:(r + 1) * P],
                                rt[:, ks, :],
                                start=(g == 0),
                                stop=(g == ngrp - 1),
                                perf_mode=mybir.MatmulPerfMode.DoubleRow,
                            )
                        if r == 1:  # stage half 0 while half 1 computes
                            nc.scalar.copy(st[:, 0:2, :], psh0[:])
                        elif r == 2 and last:
                            # per-chunk staging: chunk 2 copies while
                            # chunk 3's matmuls run, shortening the tail
                            nc.scalar.copy(st[:, 2, :], psh1[:, 0, :])
                    if last:
                        nc.scalar.copy(st[:, 3, :], psh1[:, 1, :])
                    else:
                        nc.scalar.copy(st[:, 2:4, :], psh1[:])
                    # the last two blocks bypass racc (their row maxes go
                    # through the fold tree below) so the only DVE work
                    # left after the final matmul is one block's epilogue
                    direct = (a == NA - 1 and w >= wmax - 1)

                    if not direct:
                        # running row-max (f16 2x tensor_tensor)
                        nc.vector.tensor_max(out=racc[a][:],
                                             in0=racc[a][:], in1=st[:])
                    else:
                        # last two blocks bypass racc: j-fold st (must
                        # read it before the in-place colmax folds below)
                        nc.vector.tensor_max(out=stj[:],
                                             in0=st[:, :, 0:NBLK // 2],
                                             in1=st[:, :, NBLK // 2:NBLK])
                    # column-max fold to [128, 512], in place into st
                    # chunks 0-1 then 0 (racc/fold-tree consumed st
                    # beforehand); host folds the 128 partitions
                    nc.vector.tensor_max(out=st[:, 0:2, :],
                                         in0=st[:, 0:2, :],
                                         in1=st[:, 2:4, :])
                    nc.vector.tensor_max(out=st[:, 0, :],
                                         in0=st[:, 0, :],
                                         in1=st[:, 1, :])
                    nc.sync.dma_start(
                        colmax_ap[sb_idx * P:(sb_idx + 1) * P, :],
                        st[:, 0, :])
                    if direct:
                        # small reduce + merge (racc for this window was
                        # reduced after w = wmax-2, two blocks of matmul
                        # cover before the kernel ends)
                        nc.vector.tensor_max(out=stj2[:],
                                             in0=stj[:, :, 0:NBLK // 4],
                                             in1=stj[:, :, NBLK // 4:NBLK // 2])
                        row_reduce(rowtmp[:], stj2)
                        nc.vector.tensor_max(
                            out=rowmax_sb[:, 4 * a:4 * a + 4],
                            in0=rowmax_sb[:, 4 * a:4 * a + 4],
                            in1=rowtmp[:])
                sb_idx += 1

                # per-window row-max reduction (hidden under the next
                # block's matmuls; for the very last window it runs two
                # blocks early — the DVE lags the PE by about a block, so
                # anything issued later still lands after the final MM)
                red_now = (w == wmax - 2) if (a == NA - 1) else (w == wmax)
                if red_now:
                    row_reduce(rowmax_sb[:, 4 * a:4 * a + 4], racc[a])

            nc.sync.dma_start(rowmax_d[:], rowmax_sb[:])

    nc.compile()
    return nc


def prepare_inputs(x):
    """Host prep: normalize (f64), scale+cast fp8, retile, rotate."""
    xd = np.asarray(x, dtype=np.float64)
    norms = np.sqrt(np.einsum("ij,ij->i", xd, xd))
    np.maximum(norms, 1e-12, out=norms)
    xn = xd / norms[:, None]
    xnt = (xn.T * FP8_SCALE).astype(ml_dtypes.float8_e4m3)  # [D, B]
    # retile to [J, p, k, j]: xt_r[J, p, k, j] = xnt[k*128+p, J*512+j]
    xt_r = np.ascontiguousarray(
        xnt.reshape(KCH, P, NSB, NBLK).transpose(2, 1, 0, 3))
    eyef16 = np.ascontiguousarray(
        EYE_VAL * np.eye(P, dtype=np.float32)).astype(np.float16)
    in_maps = []
    for c in range(N_CORES):
        rot = (np.concatenate([xt_r[c:], xt_r[:c]], axis=0) if c
               else xt_r)
        in_maps.append({
            "xt": np.ascontiguousarray(rot).reshape(NSB * P, KCH * NBLK),
            "eyef16": eyef16,
        })
    return in_maps


def postprocess(results):
    """Merge per-core row/column maxima and apply the scalar epilogue."""
    inv = 1.0 / (FP8_SCALE * FP8_SCALE)
    order = _windows()
    maxsim = np.full(B, -np.inf, dtype=np.float64)
    for c in range(N_CORES):
        rm = np.asarray(results[c]["rowmax"], dtype=np.float64)  # [P, 16]
        for a in range(NA):
            for r in range(4):
                g0 = (c + 8 * a) * NBLK + r * P  # global row of partition 0
                sl = slice(g0, g0 + P)
                np.maximum(maxsim[sl], rm[:, 4 * a + r], out=maxsim[sl])
        cmx = np.asarray(results[c]["colmax"]).astype(np.float32)
        cmx = cmx.reshape(N_SLOTS, P, NBLK).max(axis=1).astype(np.float64)
        for s, (a, w) in enumerate(order):
            g0 = ((8 * a + w + c) % NSB) * NBLK
            if w == 0:
                # diag slot: cols 0:384 cover block cols 128..512
                sl = slice(g0 + P, g0 + NBLK)
                np.maximum(maxsim[sl], cmx[s, 0:DIAG_W], out=maxsim[sl])
            else:
                sl = slice(g0, g0 + NBLK)
                np.maximum(maxsim[sl], cmx[s], out=maxsim[sl])
    d2 = 2.0 - 2.0 * (maxsim * inv) + EPS
    loss = -0.5 * np.mean(np.log(d2))
    return np.array(loss, dtype=np.float32)


_NC_CACHE = {}


def _get_nc():
    if "nc" not in _NC_CACHE:
        _NC_CACHE["nc"] = build_nc()
    return _NC_CACHE["nc"]


def kernel(x, **_ignored):
    import time

    nc = _get_nc()
    in_maps = prepare_inputs(x)
    last_exc = None
    for attempt in range(3):
        try:
            res = run_bass_kernel_spmd(nc, in_maps,
                                       core_ids=list(range(N_CORES)))
            return postprocess(res.results)
        except Exception as exc:  # transient NRT/tunnel hiccups
            last_exc = exc
            if attempt < 2:
                time.sleep(30)  # a wedged exec unit takes a while to heal
    raise last_exc


if __name__ == "__main__":
    x = np.random.default_rng(0).standard_normal((B, D), dtype=np.float32)
    print(kernel(x))


# revision 34
# speedup vs baseline: 1.0531x; 1.0531x over previous
"""KoLeo loss kernel for 8 Trainium2 NeuronCores — symmetric (half-matrix)
variant, v2.

Reference computation (B=16384, D=1024):
    xn  = x / max(||x||_2, 1e-12)          # row L2-normalize
    sim = xn @ xn.T                        # B x B cosine similarity
    max_sim[i] = max_{j != i} sim[i, j]    # nearest neighbor (excl. self)
    out = -mean(log(sqrt(2 - 2*max_sim + 1e-8)))

sim is symmetric, so only the upper triangle of 512x512 super-blocks is
computed. Work distribution ("pencil window", SPMD-uniform): global
super-row G is owned by core c = G % 8. Each core holds its 4 owned
super-rows resident and computes super-blocks (I, I+w mod 32) for
w = 0..16 (a in {0,1}) or w = 0..15 (a in {2,3}); every unordered pair
of super-blocks is covered exactly once across the fleet (528 total).

Measured (8-core SPMD, core 0 profile): ~249us on a 2.4 GHz run,
~296us when the chip's P0 power limit drops the PE to ~2.0 GHz
(uncontrollable, ~25% of runs). Breakdown of a warm run: ~7us engine
preamble, first matmul at ~9us, 225.8us matmul stream (99% dense,
216 ns per 512-wide fp8 DoubleRow matmul = the 2.4 GHz streaming
roofline), ~4us epilogue tail, ~10us framework teardown.

Changes vs v1 (which ran PE and DVE both ~90% busy at ~3.85us/block,
258-308us total depending on the chip power state):
  - Diagonal (w=0) super-blocks only compute their upper-triangle
    128-row chunks (rows chunk r x cols 128r..512): 5120 instead of
    8192 PE cycles per diag block. A ragged column-max tile (cols
    128..512, max over the computed rows, self-sims masked to -130+64)
    is shipped so the skipped lower-triangle pairs stay covered.
  - DVE row-max restructure: instead of a full 2.2us tensor_reduce per
    block, a running row-max accumulator racc[a] takes one f16 2x-mode
    tensor_max per block (~1.1us); the expensive reduce happens once
    per window. DVE drops from ~3.85us/block (co-bottleneck with PE)
    to ~2.1us/block. (tensor_tensor_reduce would fuse this further but
    faults the exec unit on this runtime — verified by HW probe.)
  - rhs blocks at local index 8k (w = 8, 16) are the resident lhs
    tiles — 6 fewer 512KB HBM reads.
  - All DMA issues move off the Scalar queue (ACT was doing 44us of
    DMA_DIRECT2D issue on top of its 156us of PSUM-drain copies):
    inputs on Sync, outputs on GpSimd (otherwise idle).
  - ~24 tiny f16 warm-up matmuls on the eye tile run during the input
    DMA window so the PE's HAM clock-gate (cold 1.2 GHz for the first
    ~3.4us of activity) warms up on garbage instead of real work.

Per-block engine budget (warm, 2.4 GHz PE): PE 3.41us (16 fp8 DoubleRow
matmuls, N=512, zero inter-MM bubble measured), ACT 2.36us (PSUM->f16
SBUF stage copy, sole bulk PSUM consumer), DVE ~2.1us, leaving PE the
sole bottleneck at ~220us/core + start/tail.

Host: pre-normalizes rows (f64), scales by 8 and casts to fp8e4m3,
pre-rotates/retiles per core; post-merges row/column maxima across
cores and applies the scalar log epilogue in f64.
"""

import sys

if "/opt/trn_rl_repo" not in sys.path:
    sys.path.insert(0, "/opt/trn_rl_repo")

import numpy as np
import ml_dtypes

import concourse.bass as bass  # noqa: F401  (import keeps bass registered)
import concourse.mybir as mybir
import concourse.tile as tile
from concourse import bacc
from concourse.bass_utils import run_bass_kernel_spmd

P = 128          # SBUF partitions
NBLK = 512       # super-block side (= one PSUM bank of f32 per 128 rows)
EPS = 1e-8

B = 16384        # rows of x
D = 1024         # feature dim
N_CORES = 8
NSB = B // NBLK  # 32 super-blocks per matrix side
KCH = D // P     # 8 contraction chunks of 128
KSTEP = 2        # fp8 DoubleRow: K chunks of 256 per matmul
NA = 4           # owned super-rows per core (global stride 8)
FP8_SCALE = 8.0
EYE_VAL = -130.0  # added at self-sim positions (value 64) before maxes
N_WARMUP = 18    # f16 eye matmuls to heat the PE HAM clock-gate


def _windows():
    """Program-order (a, w) list. w=0 is the diagonal super-block.

    a in {0,1} get w up to 16, a in {2,3} up to 15: the distance-16 pairs
    {i, i+16} are covered once by the a0/a1 windows (i = c + 8*a0), so the
    a2/a3 windows stop at 15. Total 66 super-blocks per core; the union of
    (owned I, I+w) over all cores covers every unordered block pair once.
    """
    out = []
    for a in range(NA):
        wmax = 16 if a < 2 else 15
        for w in range(wmax + 1):
            out.append((a, w))
    return out


N_SLOTS = len(_windows())  # 66: every block ships a colmax tile now
DIAG_W = 384               # diag colmax covers block cols 128..512


def build_nc():
    """Build the per-core SPMD Bass program.

    Inputs :  xt     [NSB*P, KCH*NBLK] fp8e4m3 — normalized, scaled x.T,
              retiled as [J, p, k, j] and column-rotated by 512*c so owned
              super-rows sit at local block 8a.
              eyef16 [P, P] f16 — the constant EYE_VAL * I
    Outputs:  rowmax [P, 16]          f32 — [p, 4a+r] = row-max over the
              computed window for local row 4096a + 128r + p
              colmax [N_SLOTS*P, NBLK] f16 — per super-block (program
              order), the r-chunk-folded [128, 512] column-max tile (for
              diagonal blocks only cols 0:384 are valid, covering block
              cols 128..512); the host reduces the 128 partitions.
    """
    f32 = mybir.dt.float32
    f16 = mybir.dt.float16
    fp8 = mybir.dt.float8e4
    ngrp = KCH // KSTEP

    nc = bacc.Bacc("TRN2", target_bir_lowering=False, debug=False,
                   num_devices=N_CORES)
    xt = nc.dram_tensor("xt", [NSB * P, KCH * NBLK], fp8,
                        kind="ExternalInput")
    eyed = nc.dram_tensor("eyef16", [P, P], f16, kind="ExternalInput")
    rowmax_d = nc.dram_tensor("rowmax", [P, NA * 4], f32,
                              kind="ExternalOutput")
    colmax_d = nc.dram_tensor("colmax", [N_SLOTS * P, NBLK], f16,
                              kind="ExternalOutput")
    xt_ap = xt[:]
    colmax_ap = colmax_d[:]

    with tile.TileContext(nc) as tc:
        with (
            tc.tile_pool(name="lhs", bufs=1) as lhs_pool,
            tc.tile_pool(name="rhs", bufs=3) as rhs_pool,
            tc.tile_pool(name="psum", bufs=4, space="PSUM") as psum_pool,
            tc.tile_pool(name="stage", bufs=4) as stage_pool,
            tc.tile_pool(name="stats", bufs=1) as stats_pool,
        ):
            # --- input DMAs, all on the Sync queue. Order matters: lhs0
            # first (the first super-block needs only it), then the eye
            # (not consumed until the first diagonal epilogue); lhs1..3
            # are issued from inside the loop so early rhs prefetches
            # aren't stuck behind them.
            eye = stats_pool.tile([P, P], f16, name="eye")

            lhs_tiles = [
                lhs_pool.tile([P, KCH, NBLK], fp8, name=f"lhs{a}",
                              tag=f"lhs{a}")
                for a in range(NA)
            ]

            def load_lhs(a):
                rows = slice(8 * a * P, (8 * a + 1) * P)
                nc.sync.dma_start(lhs_tiles[a][:], xt_ap[rows, :])

            load_lhs(0)
            nc.sync.dma_start(eye[:], eyed[:])

            racc = [
                stats_pool.tile([P, 4, NBLK], f16, name=f"racc{a}",
                                tag=f"racc{a}")
                for a in range(NA)
            ]
            rowmax_sb = stats_pool.tile([P, NA * 4], f32, name="rowmax_sb")
            rowtmp = stats_pool.tile([P, 4], f32, name="rowtmp")
            # scratch for the last block's j-fold tree
            stj = stats_pool.tile([P, 4, NBLK // 2], f16, name="stj")
            stj2 = stats_pool.tile([P, 4, NBLK // 4], f16, name="stj2")

            # --- PE warm-up: tiny matmuls on a never-written SBUF
            # scratch tile (garbage data, zero input dependencies — they
            # start the moment the Tensor queue is up, before any DMA
            # lands) fill the HAM activity window while lhs0 streams in.
            # They write into the first block's pre-allocated psum half;
            # its first real matmul's start=True clears has_written. ---
            wsc = stats_pool.tile([P, P], f16, name="wsc")
            nc.vector.memset(wsc[:], 0.25)
            first_ps = psum_pool.tile([P, 2, NBLK], f32, name="ps",
                                      tag="ps")
            for _ in range(N_WARMUP):
                nc.tensor.matmul(first_ps[:, 0, 0:P], wsc[:], wsc[:],
                                 start=True, stop=True)

            def row_reduce(dst_ap, src_tile):
                """dst[:, 0:4] = per-chunk row max of src [P, 4, NBLK]."""
                nc.vector.reduce_max(
                    out=dst_ap,
                    in_=src_tile[:],
                    axis=mybir.AxisListType.X,
                    op=mybir.AluOpType.max,
                )

            sb_idx = 0
            # colmax DMA issues ride the Scalar queue, deferred by one
            # block: the issue for block w waits on its DVE folds, so
            # emitting it after block w+1's ACT copies keeps the PSUM
            # drain path unblocked (Vector cannot issue DMAs; GpSimd
            # DMA rings pay a ~6.5us software drain at teardown; Sync
            # issues would head-of-line-block the rhs prefetches).
            pending_dma = []

            def flush_colmax():
                while pending_dma:
                    dst, src_ = pending_dma.pop(0)
                    nc.scalar.dma_start(dst, src_)

            for a, w in _windows():
                L = (8 * a + w) % NSB
                wmax = 16 if a < 2 else 15
                if L % 8 == 0:
                    rt = lhs_tiles[L // 8]  # resident (w = 0, 8, 16)
                else:
                    rt = rhs_pool.tile([P, KCH, NBLK], fp8, name="rt",
                                       tag="rt")
                    nc.sync.dma_start(rt[:], xt_ap[L * P:(L + 1) * P, :])
                if a == 0 and 1 <= w <= 3:
                    load_lhs(w)  # deferred so early rhs loads go first

                # two 2-bank psum halves per block: ACT staging starts
                # after 8 matmuls instead of 16 and the PE can run two
                # blocks ahead of the epilogue (4 pool bufs = 8 banks)
                if first_ps is not None:
                    psh0, first_ps = first_ps, None
                else:
                    psh0 = psum_pool.tile([P, 2, NBLK], f32, name="ps",
                                          tag="ps")
                st = stage_pool.tile([P, 4, NBLK], f16, name="st", tag="st")
                if w == 0:
                    # diagonal: chunk r covers block cols 128r..512 only
                    psh1 = None
                    for r in range(4):
                        if r == 2:
                            psh1 = psum_pool.tile([P, 2, NBLK], f32,
                                                  name="ps", tag="ps")
                        ph = psh0 if r < 2 else psh1
                        nw = NBLK - r * P
                        for g in range(ngrp):
                            ks = slice(KSTEP * g, KSTEP * (g + 1))
                            nc.tensor.matmul(
                                ph[:, r % 2, 0:nw],
                                lhs_tiles[a][:, ks, r * P:(r + 1) * P],
                                lhs_tiles[a][:, ks, r * P:NBLK],
                                start=(g == 0),
                                stop=(g == ngrp - 1),
                                perf_mode=mybir.MatmulPerfMode.DoubleRow,
                            )
                        if r == 1:  # stage half 0 while half 1 computes
                            nc.scalar.copy(st[:, 0, :], psh0[:, 0, :])
                            nc.scalar.copy(st[:, 1, 0:NBLK - P],
                                           psh0[:, 1, 0:NBLK - P])
                    nc.scalar.copy(st[:, 2, 0:NBLK - 2 * P],
                                   psh1[:, 0, 0:NBLK - 2 * P])
                    nc.scalar.copy(st[:, 3, 0:NBLK - 3 * P],
                                   psh1[:, 1, 0:NBLK - 3 * P])
                    flush_colmax()
                    # self-sim sits at st[p, r, p]: mask it below any
                    # real similarity before any max consumes it
                    for r in range(4):
                        sl = st[:, r, 0:P]
                        nc.vector.tensor_add(out=sl, in0=sl, in1=eye[:])
                    # init racc: ragged copy + -inf tails
                    nc.vector.tensor_copy(racc[a][:, 0, :], st[:, 0, :])
                    for r in range(1, 4):
                        nw = NBLK - r * P
                        nc.vector.tensor_copy(racc[a][:, r, 0:nw],
                                              st[:, r, 0:nw])
                        nc.vector.memset(racc[a][:, r, nw:NBLK], -60000.0)
                    # ragged column-max over the computed rows, block
                    # cols 128..512 (col offset 128r+j in chunk r maps to
                    # local j after the per-chunk 128-col shift), folded
                    # in place into st chunk 0 (racc copies ran already)
                    nc.vector.tensor_max(out=st[:, 0, P:NBLK],
                                         in0=st[:, 0, P:NBLK],
                                         in1=st[:, 1, 0:DIAG_W])
                    nc.vector.tensor_max(out=st[:, 0, 2 * P:NBLK],
                                         in0=st[:, 0, 2 * P:NBLK],
                                         in1=st[:, 2, 0:NBLK - 2 * P])
                    nc.vector.tensor_max(out=st[:, 0, 3 * P:NBLK],
                                         in0=st[:, 0, 3 * P:NBLK],
                                         in1=st[:, 3, 0:P])
                    pending_dma.append(
                        (colmax_ap[sb_idx * P:(sb_idx + 1) * P, 0:DIAG_W],
                         st[:, 0, P:NBLK]))
                else:
                    last = (a == NA - 1 and w == wmax)
                    psh1 = None
                    for r in range(4):
                        if r == 2:
                            psh1 = psum_pool.tile([P, 2, NBLK], f32,
                                                  name="ps", tag="ps")
                        ph = psh0 if r < 2 else psh1
                        for g in range(ngrp):
                            ks = slice(KSTEP * g, KSTEP * (g + 1))
                            nc.tensor.matmul(
                                ph[:, r % 2, :],
                                lhs_tiles[a][:, ks, r * P:(r + 1) * P],
                                rt[:, ks, :],
                                start=(g == 0),
                                stop=(g == ngrp - 1),
                                perf_mode=mybir.MatmulPerfMode.DoubleRow,
                            )
                        if r == 1:  # stage half 0 while half 1 computes
                            nc.scalar.copy(st[:, 0:2, :], psh0[:])
                        elif r == 2 and last:
                            # per-chunk staging: chunk 2 copies while
                            # chunk 3's matmuls run, shortening the tail
                            nc.scalar.copy(st[:, 2, :], psh1[:, 0, :])
                    if last:
                        nc.scalar.copy(st[:, 3, :], psh1[:, 1, :])
                    else:
                        nc.scalar.copy(st[:, 2:4, :], psh1[:])
                    flush_colmax()
                    # the last two blocks bypass racc (their row maxes go
                    # through the fold tree below) so the only DVE work
                    # left after the final matmul is one block's epilogue
                    direct = (a == NA - 1 and w >= wmax - 1)

                    if not direct:
                        # running row-max (f16 2x tensor_tensor)
                        nc.vector.tensor_max(out=racc[a][:],
                                             in0=racc[a][:], in1=st[:])
                    else:
                        # last two blocks bypass racc: j-fold st (must
                        # read it before the in-place colmax folds below)
                        nc.vector.tensor_max(out=stj[:],
                                             in0=st[:, :, 0:NBLK // 2],
                                             in1=st[:, :, NBLK // 2:NBLK])
                    # column-max fold to [128, 512], in place into st
                    # chunks 0-1 then 0 (racc/fold-tree consumed st
                    # beforehand); host folds the 128 partitions
                    nc.vector.tensor_max(out=st[:, 0:2, :],
                                         in0=st[:, 0:2, :],
                                         in1=st[:, 2:4, :])
                    nc.vector.tensor_max(out=st[:, 0, :],
                                         in0=st[:, 0, :],
                                         in1=st[:, 1, :])
                    pending_dma.append(
                        (colmax_ap[sb_idx * P:(sb_idx + 1) * P, :],
                         st[:, 0, :]))
                    if direct:
                        # small reduce + merge (racc for this window was
                        # reduced after w = wmax-2, two blocks of matmul
                        # cover before the kernel ends)
                        nc.vector.tensor_max(out=stj2[:],
                                             in0=stj[:, :, 0:NBLK // 4],
                                             in1=stj[:, :, NBLK // 4:NBLK // 2])
                        row_reduce(rowtmp[:], stj2)
                        nc.vector.tensor_max(
                            out=rowmax_sb[:, 4 * a:4 * a + 4],
                            in0=rowmax_sb[:, 4 * a:4 * a + 4],
                            in1=rowtmp[:])
                sb_idx += 1

                # per-window row-max reduction (hidden under the next
                # block's matmuls; for the very last window it runs two
                # blocks early — the DVE lags the PE by about a block, so
                # anything issued later still lands after the final MM)
                red_now = (w == wmax - 2) if (a == NA - 1) else (w == wmax)
                if red_now:
                    row_reduce(rowmax_sb[:, 4 * a:4 * a + 4], racc[a])

            flush_colmax()
            nc.sync.dma_start(rowmax_d[:], rowmax_sb[:])

    nc.compile()
    return nc


def prepare_inputs(x):
    """Host prep: normalize (f64), scale+cast fp8, retile, rotate."""
    xd = np.asarray(x, dtype=np.float64)
    norms = np.sqrt(np.einsum("ij,ij->i", xd, xd))
    np.maximum(norms, 1e-12, out=norms)
    xn = xd / norms[:, None]
    xnt = (xn.T * FP8_SCALE).astype(ml_dtypes.float8_e4m3)  # [D, B]
    # retile to [J, p, k, j]: xt_r[J, p, k, j] = xnt[k*128+p, J*512+j]
    xt_r = np.ascontiguousarray(
        xnt.reshape(KCH, P, NSB, NBLK).transpose(2, 1, 0, 3))
    eyef16 = np.ascontiguousarray(
        EYE_VAL * np.eye(P, dtype=np.float32)).astype(np.float16)
    in_maps = []
    for c in range(N_CORES):
        rot = (np.concatenate([xt_r[c:], xt_r[:c]], axis=0) if c
               else xt_r)
        in_maps.append({
            "xt": np.ascontiguousarray(rot).reshape(NSB * P, KCH * NBLK),
            "eyef16": eyef16,
        })
    return in_maps


def postprocess(results):
    """Merge per-core row/column maxima and apply the scalar epilogue."""
    inv = 1.0 / (FP8_SCALE * FP8_SCALE)
    order = _windows()
    maxsim = np.full(B, -np.inf, dtype=np.float64)
    for c in range(N_CORES):
        rm = np.asarray(results[c]["rowmax"], dtype=np.float64)  # [P, 16]
        for a in range(NA):
            for r in range(4):
                g0 = (c + 8 * a) * NBLK + r * P  # global row of partition 0
                sl = slice(g0, g0 + P)
                np.maximum(maxsim[sl], rm[:, 4 * a + r], out=maxsim[sl])
        cmx = np.asarray(results[c]["colmax"]).astype(np.float32)
        cmx = cmx.reshape(N_SLOTS, P, NBLK).max(axis=1).astype(np.float64)
        for s, (a, w) in enumerate(order):
            g0 = ((8 * a + w + c) % NSB) * NBLK
            if w == 0:
                # diag slot: cols 0:384 cover block cols 128..512
                sl = slice(g0 + P, g0 + NBLK)
                np.maximum(maxsim[sl], cmx[s, 0:DIAG_W], out=maxsim[sl])
            else:
                sl = slice(g0, g0 + NBLK)
                np.maximum(maxsim[sl], cmx[s], out=maxsim[sl])
    d2 = 2.0 - 2.0 * (maxsim * inv) + EPS
    loss = -0.5 * np.mean(np.log(d2))
    return np.array(loss, dtype=np.float32)


_NC_CACHE = {}


def _get_nc():
    if "nc" not in _NC_CACHE:
        _NC_CACHE["nc"] = build_nc()
    return _NC_CACHE["nc"]


def kernel(x, **_ignored):
    import time

    nc = _get_nc()
    in_maps = prepare_inputs(x)
    last_exc = None
    for attempt in range(3):
        try:
            res = run_bass_kernel_spmd(nc, in_maps,
                                       core_ids=list(range(N_CORES)))
            return postprocess(res.results)
        except Exception as exc:  # transient NRT/tunnel hiccups
            last_exc = exc
            if attempt < 2:
                time.sleep(30)  # a wedged exec unit takes a while to heal
    raise last_exc


if __name__ == "__main__":
    x = np.random.default_rng(0).standard_normal((B, D), dtype=np.float32)
    print(kernel(x))


# revision 35
# speedup vs baseline: 1.0597x; 1.0063x over previous
"""KoLeo loss kernel for 8 Trainium2 NeuronCores — symmetric (half-matrix)
variant, v2.

Reference computation (B=16384, D=1024):
    xn  = x / max(||x||_2, 1e-12)          # row L2-normalize
    sim = xn @ xn.T                        # B x B cosine similarity
    max_sim[i] = max_{j != i} sim[i, j]    # nearest neighbor (excl. self)
    out = -mean(log(sqrt(2 - 2*max_sim + 1e-8)))

sim is symmetric, so only the upper triangle of 512x512 super-blocks is
computed. Work distribution ("pencil window", SPMD-uniform): global
super-row G is owned by core c = G % 8. Each core holds its 4 owned
super-rows resident and computes super-blocks (I, I+w mod 32) for
w = 0..16 (a in {0,1}) or w = 0..15 (a in {2,3}); every unordered pair
of super-blocks is covered exactly once across the fleet (528 total).

Measured (8-core SPMD, core 0 profile): ~249us on a 2.4 GHz run,
~296us when the chip's P0 power limit drops the PE to ~2.0 GHz
(uncontrollable, ~25% of runs). Breakdown of a warm run: ~7us engine
preamble, first matmul at ~9us, 225.8us matmul stream (99% dense,
216 ns per 512-wide fp8 DoubleRow matmul = the 2.4 GHz streaming
roofline), ~4us epilogue tail, ~10us framework teardown.

Changes vs v1 (which ran PE and DVE both ~90% busy at ~3.85us/block,
258-308us total depending on the chip power state):
  - Diagonal (w=0) super-blocks only compute their upper-triangle
    128-row chunks (rows chunk r x cols 128r..512): 5120 instead of
    8192 PE cycles per diag block. A ragged column-max tile (cols
    128..512, max over the computed rows, self-sims masked to -130+64)
    is shipped so the skipped lower-triangle pairs stay covered.
  - DVE row-max restructure: instead of a full 2.2us tensor_reduce per
    block, a running row-max accumulator racc[a] takes one f16 2x-mode
    tensor_max per block (~1.1us); the expensive reduce happens once
    per window. DVE drops from ~3.85us/block (co-bottleneck with PE)
    to ~2.1us/block. (tensor_tensor_reduce would fuse this further but
    faults the exec unit on this runtime — verified by HW probe.)
  - rhs blocks at local index 8k (w = 8, 16) are the resident lhs
    tiles — 6 fewer 512KB HBM reads.
  - All DMA issues move off the Scalar queue (ACT was doing 44us of
    DMA_DIRECT2D issue on top of its 156us of PSUM-drain copies):
    inputs on Sync, outputs on GpSimd (otherwise idle).
  - ~24 tiny f16 warm-up matmuls on the eye tile run during the input
    DMA window so the PE's HAM clock-gate (cold 1.2 GHz for the first
    ~3.4us of activity) warms up on garbage instead of real work.

Per-block engine budget (warm, 2.4 GHz PE): PE 3.41us (16 fp8 DoubleRow
matmuls, N=512, zero inter-MM bubble measured), ACT 2.36us (PSUM->f16
SBUF stage copy, sole bulk PSUM consumer), DVE ~2.1us, leaving PE the
sole bottleneck at ~220us/core + start/tail.

Host: pre-normalizes rows (f64), scales by 8 and casts to fp8e4m3,
pre-rotates/retiles per core; post-merges row/column maxima across
cores and applies the scalar log epilogue in f64.
"""

import sys

if "/opt/trn_rl_repo" not in sys.path:
    sys.path.insert(0, "/opt/trn_rl_repo")

import numpy as np
import ml_dtypes

import concourse.bass as bass  # noqa: F401  (import keeps bass registered)
import concourse.mybir as mybir
import concourse.tile as tile
from concourse import bacc
from concourse.bass_utils import run_bass_kernel_spmd

P = 128          # SBUF partitions
NBLK = 512       # super-block side (= one PSUM bank of f32 per 128 rows)
EPS = 1e-8

B = 16384        # rows of x
D = 1024         # feature dim
N_CORES = 8
NSB = B // NBLK  # 32 super-blocks per matrix side
KCH = D // P     # 8 contraction chunks of 128
KSTEP = 2        # fp8 DoubleRow: K chunks of 256 per matmul
NA = 4           # owned super-rows per core (global stride 8)
FP8_SCALE = 8.0
EYE_VAL = -130.0  # added at self-sim positions (value 64) before maxes
N_WARMUP = 18    # f16 eye matmuls to heat the PE HAM clock-gate


def _windows():
    """Program-order (a, w) list. w=0 is the diagonal super-block.

    a in {0,1} get w up to 16, a in {2,3} up to 15: the distance-16 pairs
    {i, i+16} are covered once by the a0/a1 windows (i = c + 8*a0), so the
    a2/a3 windows stop at 15. Total 66 super-blocks per core; the union of
    (owned I, I+w) over all cores covers every unordered block pair once.
    """
    out = []
    for a in range(NA):
        wmax = 16 if a < 2 else 15
        for w in range(wmax + 1):
            out.append((a, w))
    return out


N_SLOTS = len(_windows())  # 66: every block ships a colmax tile now
DIAG_W = 384               # diag colmax covers block cols 128..512


def build_nc():
    """Build the per-core SPMD Bass program.

    Inputs :  xt     [NSB*P, KCH*NBLK] fp8e4m3 — normalized, scaled x.T,
              retiled as [J, p, k, j] and column-rotated by 512*c so owned
              super-rows sit at local block 8a.
              eyef16 [P, P] f16 — the constant EYE_VAL * I
    Outputs:  rowmax [P, 16]          f32 — [p, 4a+r] = row-max over the
              computed window for local row 4096a + 128r + p
              colmax [N_SLOTS*P, NBLK] f16 — per super-block (program
              order), the r-chunk-folded [128, 512] column-max tile (for
              diagonal blocks only cols 0:384 are valid, covering block
              cols 128..512); the host reduces the 128 partitions.
    """
    f32 = mybir.dt.float32
    f16 = mybir.dt.float16
    fp8 = mybir.dt.float8e4
    ngrp = KCH // KSTEP

    nc = bacc.Bacc("TRN2", target_bir_lowering=False, debug=False,
                   num_devices=N_CORES)
    xt = nc.dram_tensor("xt", [NSB * P, KCH * NBLK], fp8,
                        kind="ExternalInput")
    eyed = nc.dram_tensor("eyef16", [P, P], f16, kind="ExternalInput")
    rowmax_d = nc.dram_tensor("rowmax", [P, NA * 4], f32,
                              kind="ExternalOutput")
    colmax_d = nc.dram_tensor("colmax", [N_SLOTS * P, NBLK], f16,
                              kind="ExternalOutput")
    xt_ap = xt[:]
    colmax_ap = colmax_d[:]

    with tile.TileContext(nc) as tc:
        with (
            tc.tile_pool(name="lhs", bufs=1) as lhs_pool,
            tc.tile_pool(name="rhs", bufs=4) as rhs_pool,
            tc.tile_pool(name="psum", bufs=4, space="PSUM") as psum_pool,
            tc.tile_pool(name="stage", bufs=5) as stage_pool,
            tc.tile_pool(name="stats", bufs=1) as stats_pool,
        ):
            # --- input DMAs, all on the Sync queue. Order matters: lhs0
            # first (the first super-block needs only it), then the eye
            # (not consumed until the first diagonal epilogue); lhs1..3
            # are issued from inside the loop so early rhs prefetches
            # aren't stuck behind them.
            eye = stats_pool.tile([P, P], f16, name="eye")

            lhs_tiles = [
                lhs_pool.tile([P, KCH, NBLK], fp8, name=f"lhs{a}",
                              tag=f"lhs{a}")
                for a in range(NA)
            ]

            def load_lhs(a):
                rows = slice(8 * a * P, (8 * a + 1) * P)
                nc.sync.dma_start(lhs_tiles[a][:], xt_ap[rows, :])

            load_lhs(0)
            nc.sync.dma_start(eye[:], eyed[:])

            racc = [
                stats_pool.tile([P, 4, NBLK], f16, name=f"racc{a}",
                                tag=f"racc{a}")
                for a in range(NA)
            ]
            rowmax_sb = stats_pool.tile([P, NA * 4], f32, name="rowmax_sb")
            rowtmp = stats_pool.tile([P, 4], f32, name="rowtmp")
            # scratch for the last block's j-fold tree
            stj = stats_pool.tile([P, 4, NBLK // 2], f16, name="stj")
            stj2 = stats_pool.tile([P, 4, NBLK // 4], f16, name="stj2")

            # --- PE warm-up: tiny matmuls on a never-written SBUF
            # scratch tile (garbage data, zero input dependencies — they
            # start the moment the Tensor queue is up, before any DMA
            # lands) fill the HAM activity window while lhs0 streams in.
            # They write into the first block's pre-allocated psum half;
            # its first real matmul's start=True clears has_written. ---
            wsc = stats_pool.tile([P, P], f16, name="wsc")
            nc.vector.memset(wsc[:], 0.25)
            first_ps = psum_pool.tile([P, 2, NBLK], f32, name="ps",
                                      tag="ps")
            for _ in range(N_WARMUP):
                nc.tensor.matmul(first_ps[:, 0, 0:P], wsc[:], wsc[:],
                                 start=True, stop=True)

            def row_reduce(dst_ap, src_tile):
                """dst[:, 0:4] = per-chunk row max of src [P, 4, NBLK]."""
                nc.vector.reduce_max(
                    out=dst_ap,
                    in_=src_tile[:],
                    axis=mybir.AxisListType.X,
                    op=mybir.AluOpType.max,
                )

            sb_idx = 0
            # colmax DMA issues ride the Scalar queue, deferred by one
            # block: the issue for block w waits on its DVE folds, so
            # emitting it after block w+1's ACT copies keeps the PSUM
            # drain path unblocked (Vector cannot issue DMAs; GpSimd
            # DMA rings pay a ~6.5us software drain at teardown; Sync
            # issues would head-of-line-block the rhs prefetches).
            pending_dma = []

            def flush_colmax():
                while pending_dma:
                    dst, src_ = pending_dma.pop(0)
                    nc.scalar.dma_start(dst, src_)

            for a, w in _windows():
                L = (8 * a + w) % NSB
                wmax = 16 if a < 2 else 15
                if L % 8 == 0:
                    rt = lhs_tiles[L // 8]  # resident (w = 0, 8, 16)
                else:
                    rt = rhs_pool.tile([P, KCH, NBLK], fp8, name="rt",
                                       tag="rt")
                    nc.sync.dma_start(rt[:], xt_ap[L * P:(L + 1) * P, :])
                if a == 0 and 1 <= w <= 3:
                    load_lhs(w)  # deferred so early rhs loads go first

                # two 2-bank psum halves per block: ACT staging starts
                # after 8 matmuls instead of 16 and the PE can run two
                # blocks ahead of the epilogue (4 pool bufs = 8 banks)
                if first_ps is not None:
                    psh0, first_ps = first_ps, None
                else:
                    psh0 = psum_pool.tile([P, 2, NBLK], f32, name="ps",
                                          tag="ps")
                st = stage_pool.tile([P, 4, NBLK], f16, name="st", tag="st")
                if w == 0:
                    # diagonal: chunk r covers block cols 128r..512 only
                    psh1 = None
                    for r in range(4):
                        if r == 2:
                            psh1 = psum_pool.tile([P, 2, NBLK], f32,
                                                  name="ps", tag="ps")
                        ph = psh0 if r < 2 else psh1
                        nw = NBLK - r * P
                        for g in range(ngrp):
                            ks = slice(KSTEP * g, KSTEP * (g + 1))
                            nc.tensor.matmul(
                                ph[:, r % 2, 0:nw],
                                lhs_tiles[a][:, ks, r * P:(r + 1) * P],
                                lhs_tiles[a][:, ks, r * P:NBLK],
                                start=(g == 0),
                                stop=(g == ngrp - 1),
                                perf_mode=mybir.MatmulPerfMode.DoubleRow,
                            )
                        if r == 1:  # stage half 0 while half 1 computes
                            nc.scalar.copy(st[:, 0, :], psh0[:, 0, :])
                            nc.scalar.copy(st[:, 1, 0:NBLK - P],
                                           psh0[:, 1, 0:NBLK - P])
                    nc.scalar.copy(st[:, 2, 0:NBLK - 2 * P],
                                   psh1[:, 0, 0:NBLK - 2 * P])
                    nc.scalar.copy(st[:, 3, 0:NBLK - 3 * P],
                                   psh1[:, 1, 0:NBLK - 3 * P])
                    flush_colmax()
                    # self-sim sits at st[p, r, p]: mask it below any
                    # real similarity before any max consumes it
                    for r in range(4):
                        sl = st[:, r, 0:P]
                        nc.vector.tensor_add(out=sl, in0=sl, in1=eye[:])
                    # init racc: ragged copy + -inf tails
                    nc.vector.tensor_copy(racc[a][:, 0, :], st[:, 0, :])
                    for r in range(1, 4):
                        nw = NBLK - r * P
                        nc.vector.tensor_copy(racc[a][:, r, 0:nw],
                                              st[:, r, 0:nw])
                        nc.vector.memset(racc[a][:, r, nw:NBLK], -60000.0)
                    # ragged column-max over the computed rows, block
                    # cols 128..512 (col offset 128r+j in chunk r maps to
                    # local j after the per-chunk 128-col shift), folded
                    # in place into st chunk 0 (racc copies ran already)
                    nc.vector.tensor_max(out=st[:, 0, P:NBLK],
                                         in0=st[:, 0, P:NBLK],
                                         in1=st[:, 1, 0:DIAG_W])
                    nc.vector.tensor_max(out=st[:, 0, 2 * P:NBLK],
                                         in0=st[:, 0, 2 * P:NBLK],
                                         in1=st[:, 2, 0:NBLK - 2 * P])
                    nc.vector.tensor_max(out=st[:, 0, 3 * P:NBLK],
                                         in0=st[:, 0, 3 * P:NBLK],
                                         in1=st[:, 3, 0:P])
                    pending_dma.append(
                        (colmax_ap[sb_idx * P:(sb_idx + 1) * P, 0:DIAG_W],
                         st[:, 0, P:NBLK]))
                else:
                    last = (a == NA - 1 and w == wmax)
                    psh1 = None
                    for r in range(4):
                        if r == 2:
                            psh1 = psum_pool.tile([P, 2, NBLK], f32,
                                                  name="ps", tag="ps")
                        ph = psh0 if r < 2 else psh1
                        for g in range(ngrp):
                            ks = slice(KSTEP * g, KSTEP * (g + 1))
                            nc.tensor.matmul(
                                ph[:, r % 2, :],
                                lhs_tiles[a][:, ks, r * P:(r + 1) * P],
                                rt[:, ks, :],
                                start=(g == 0),
                                stop=(g == ngrp - 1),
                                perf_mode=mybir.MatmulPerfMode.DoubleRow,
                            )
                        if r == 1:  # stage half 0 while half 1 computes
                            nc.scalar.copy(st[:, 0:2, :], psh0[:])
                        elif r == 2 and last:
                            # per-chunk staging: chunk 2 copies while
                            # chunk 3's matmuls run, shortening the tail
                            nc.scalar.copy(st[:, 2, :], psh1[:, 0, :])
                    if last:
                        nc.scalar.copy(st[:, 3, :], psh1[:, 1, :])
                    else:
                        nc.scalar.copy(st[:, 2:4, :], psh1[:])
                    flush_colmax()
                    # the last two blocks bypass racc (their row maxes go
                    # through the fold tree below) so the only DVE work
                    # left after the final matmul is one block's epilogue
                    direct = (a == NA - 1 and w >= wmax - 1)

                    if not direct:
                        # running row-max (f16 2x tensor_tensor)
                        nc.vector.tensor_max(out=racc[a][:],
                                             in0=racc[a][:], in1=st[:])
                    else:
                        # last two blocks bypass racc: j-fold st (must
                        # read it before the in-place colmax folds below)
                        nc.vector.tensor_max(out=stj[:],
                                             in0=st[:, :, 0:NBLK // 2],
                                             in1=st[:, :, NBLK // 2:NBLK])
                    # column-max fold to [128, 512], in place into st
                    # chunks 0-1 then 0 (racc/fold-tree consumed st
                    # beforehand); host folds the 128 partitions
                    nc.vector.tensor_max(out=st[:, 0:2, :],
                                         in0=st[:, 0:2, :],
                                         in1=st[:, 2:4, :])
                    nc.vector.tensor_max(out=st[:, 0, :],
                                         in0=st[:, 0, :],
                                         in1=st[:, 1, :])
                    pending_dma.append(
                        (colmax_ap[sb_idx * P:(sb_idx + 1) * P, :],
                         st[:, 0, :]))
                    if direct:
                        # small reduce + merge (racc for this window was
                        # reduced after w = wmax-2, two blocks of matmul
                        # cover before the kernel ends)
                        nc.vector.tensor_max(out=stj2[:],
                                             in0=stj[:, :, 0:NBLK // 4],
                                             in1=stj[:, :, NBLK // 4:NBLK // 2])
                        row_reduce(rowtmp[:], stj2)
                        nc.vector.tensor_max(
                            out=rowmax_sb[:, 4 * a:4 * a + 4],
                            in0=rowmax_sb[:, 4 * a:4 * a + 4],
                            in1=rowtmp[:])
                sb_idx += 1

                # per-window row-max reduction (hidden under the next
                # block's matmuls; for the very last window it runs two
                # blocks early — the DVE lags the PE by about a block, so
                # anything issued later still lands after the final MM)
                red_now = (w == wmax - 2) if (a == NA - 1) else (w == wmax)
                if red_now:
                    row_reduce(rowmax_sb[:, 4 * a:4 * a + 4], racc[a])

            flush_colmax()
            nc.sync.dma_start(rowmax_d[:], rowmax_sb[:])

    nc.compile()
    return nc


def prepare_inputs(x):
    """Host prep: normalize (f64), scale+cast fp8, retile, rotate."""
    xd = np.asarray(x, dtype=np.float64)
    norms = np.sqrt(np.einsum("ij,ij->i", xd, xd))
    np.maximum(norms, 1e-12, out=norms)
    xn = xd / norms[:, None]
    xnt = (xn.T * FP8_SCALE).astype(ml_dtypes.float8_e4m3)  # [D, B]
    # retile to [J, p, k, j]: xt_r[J, p, k, j] = xnt[k*128+p, J*512+j]
    xt_r = np.ascontiguousarray(
        xnt.reshape(KCH, P, NSB, NBLK).transpose(2, 1, 0, 3))
    eyef16 = np.ascontiguousarray(
        EYE_VAL * np.eye(P, dtype=np.float32)).astype(np.float16)
    in_maps = []
    for c in range(N_CORES):
        rot = (np.concatenate([xt_r[c:], xt_r[:c]], axis=0) if c
               else xt_r)
        in_maps.append({
            "xt": np.ascontiguousarray(rot).reshape(NSB * P, KCH * NBLK),
            "eyef16": eyef16,
        })
    return in_maps


def postprocess(results):
    """Merge per-core row/column maxima and apply the scalar epilogue."""
    inv = 1.0 / (FP8_SCALE * FP8_SCALE)
    order = _windows()
    maxsim = np.full(B, -np.inf, dtype=np.float64)
    for c in range(N_CORES):
        rm = np.asarray(results[c]["rowmax"], dtype=np.float64)  # [P, 16]
        for a in range(NA):
            for r in range(4):
                g0 = (c + 8 * a) * NBLK + r * P  # global row of partition 0
                sl = slice(g0, g0 + P)
                np.maximum(maxsim[sl], rm[:, 4 * a + r], out=maxsim[sl])
        cmx = np.asarray(results[c]["colmax"]).astype(np.float32)
        cmx = cmx.reshape(N_SLOTS, P, NBLK).max(axis=1).astype(np.float64)
        for s, (a, w) in enumerate(order):
            g0 = ((8 * a + w + c) % NSB) * NBLK
            if w == 0:
                # diag slot: cols 0:384 cover block cols 128..512
                sl = slice(g0 + P, g0 + NBLK)
                np.maximum(maxsim[sl], cmx[s, 0:DIAG_W], out=maxsim[sl])
            else:
                sl = slice(g0, g0 + NBLK)
                np.maximum(maxsim[sl], cmx[s], out=maxsim[sl])
    d2 = 2.0 - 2.0 * (maxsim * inv) + EPS
    loss = -0.5 * np.mean(np.log(d2))
    return np.array(loss, dtype=np.float32)


_NC_CACHE = {}


def _get_nc():
    if "nc" not in _NC_CACHE:
        _NC_CACHE["nc"] = build_nc()
    return _NC_CACHE["nc"]


def kernel(x, **_ignored):
    import time

    nc = _get_nc()
    in_maps = prepare_inputs(x)
    last_exc = None
    for attempt in range(3):
        try:
            res = run_bass_kernel_spmd(nc, in_maps,
                                       core_ids=list(range(N_CORES)))
            return postprocess(res.results)
        except Exception as exc:  # transient NRT/tunnel hiccups
            last_exc = exc
            if attempt < 2:
                time.sleep(30)  # a wedged exec unit takes a while to heal
    raise last_exc


if __name__ == "__main__":
    x = np.random.default_rng(0).standard_normal((B, D), dtype=np.float32)
    print(kernel(x))


# revision 36
# speedup vs baseline: 1.0633x; 1.0034x over previous
"""KoLeo loss kernel for 8 Trainium2 NeuronCores — symmetric (half-matrix)
variant, v2.

Reference computation (B=16384, D=1024):
    xn  = x / max(||x||_2, 1e-12)          # row L2-normalize
    sim = xn @ xn.T                        # B x B cosine similarity
    max_sim[i] = max_{j != i} sim[i, j]    # nearest neighbor (excl. self)
    out = -mean(log(sqrt(2 - 2*max_sim + 1e-8)))

sim is symmetric, so only the upper triangle of 512x512 super-blocks is
computed. Work distribution ("pencil window", SPMD-uniform): global
super-row G is owned by core c = G % 8. Each core holds its 4 owned
super-rows resident and computes super-blocks (I, I+w mod 32) for
w = 0..16 (a in {0,1}) or w = 0..15 (a in {2,3}); every unordered pair
of super-blocks is covered exactly once across the fleet (528 total).

Measured (8-core SPMD, core 0 profile): ~249us on a 2.4 GHz run,
~296us when the chip's P0 power limit drops the PE to ~2.0 GHz
(uncontrollable, ~25% of runs). Breakdown of a warm run: ~7us engine
preamble, first matmul at ~9us, 225.8us matmul stream (99% dense,
216 ns per 512-wide fp8 DoubleRow matmul = the 2.4 GHz streaming
roofline), ~4us epilogue tail, ~10us framework teardown.

Changes vs v1 (which ran PE and DVE both ~90% busy at ~3.85us/block,
258-308us total depending on the chip power state):
  - Diagonal (w=0) super-blocks only compute their upper-triangle
    128-row chunks (rows chunk r x cols 128r..512): 5120 instead of
    8192 PE cycles per diag block. A ragged column-max tile (cols
    128..512, max over the computed rows, self-sims masked to -130+64)
    is shipped so the skipped lower-triangle pairs stay covered.
  - DVE row-max restructure: instead of a full 2.2us tensor_reduce per
    block, a running row-max accumulator racc[a] takes one f16 2x-mode
    tensor_max per block (~1.1us); the expensive reduce happens once
    per window. DVE drops from ~3.85us/block (co-bottleneck with PE)
    to ~2.1us/block. (tensor_tensor_reduce would fuse this further but
    faults the exec unit on this runtime — verified by HW probe.)
  - rhs blocks at local index 8k (w = 8, 16) are the resident lhs
    tiles — 6 fewer 512KB HBM reads.
  - All DMA issues move off the Scalar queue (ACT was doing 44us of
    DMA_DIRECT2D issue on top of its 156us of PSUM-drain copies):
    inputs on Sync, outputs on GpSimd (otherwise idle).
  - ~24 tiny f16 warm-up matmuls on the eye tile run during the input
    DMA window so the PE's HAM clock-gate (cold 1.2 GHz for the first
    ~3.4us of activity) warms up on garbage instead of real work.

Per-block engine budget (warm, 2.4 GHz PE): PE 3.41us (16 fp8 DoubleRow
matmuls, N=512, zero inter-MM bubble measured), ACT 2.36us (PSUM->f16
SBUF stage copy, sole bulk PSUM consumer), DVE ~2.1us, leaving PE the
sole bottleneck at ~220us/core + start/tail.

Host: pre-normalizes rows (f64), scales by 8 and casts to fp8e4m3,
pre-rotates/retiles per core; post-merges row/column maxima across
cores and applies the scalar log epilogue in f64.
"""

import sys

if "/opt/trn_rl_repo" not in sys.path:
    sys.path.insert(0, "/opt/trn_rl_repo")

import numpy as np
import ml_dtypes

import concourse.bass as bass  # noqa: F401  (import keeps bass registered)
import concourse.mybir as mybir
import concourse.tile as tile
from concourse import bacc
from concourse.bass_utils import run_bass_kernel_spmd

P = 128          # SBUF partitions
NBLK = 512       # super-block side (= one PSUM bank of f32 per 128 rows)
EPS = 1e-8

B = 16384        # rows of x
D = 1024         # feature dim
N_CORES = 8
NSB = B // NBLK  # 32 super-blocks per matrix side
KCH = D // P     # 8 contraction chunks of 128
KSTEP = 2        # fp8 DoubleRow: K chunks of 256 per matmul
NA = 4           # owned super-rows per core (global stride 8)
FP8_SCALE = 8.0
EYE_VAL = -130.0  # added at self-sim positions (value 64) before maxes
N_WARMUP = 38    # tiny matmuls to heat the PE HAM clock-gate
                 # (sized to cover until the lhs0 DMA lands ~12us)


def _windows():
    """Program-order (a, w) list. w=0 is the diagonal super-block.

    a in {0,1} get w up to 16, a in {2,3} up to 15: the distance-16 pairs
    {i, i+16} are covered once by the a0/a1 windows (i = c + 8*a0), so the
    a2/a3 windows stop at 15. Total 66 super-blocks per core; the union of
    (owned I, I+w) over all cores covers every unordered block pair once.
    """
    out = []
    for a in range(NA):
        wmax = 16 if a < 2 else 15
        for w in range(wmax + 1):
            out.append((a, w))
    return out


N_SLOTS = len(_windows())  # 66: every block ships a colmax tile now
DIAG_W = 384               # diag colmax covers block cols 128..512


def build_nc():
    """Build the per-core SPMD Bass program.

    Inputs :  xt     [NSB*P, KCH*NBLK] fp8e4m3 — normalized, scaled x.T,
              retiled as [J, p, k, j] and column-rotated by 512*c so owned
              super-rows sit at local block 8a.
              eyef16 [P, P] f16 — the constant EYE_VAL * I
    Outputs:  rowmax [P, 16]          f32 — [p, 4a+r] = row-max over the
              computed window for local row 4096a + 128r + p
              colmax [N_SLOTS*P, NBLK] f16 — per super-block (program
              order), the r-chunk-folded [128, 512] column-max tile (for
              diagonal blocks only cols 0:384 are valid, covering block
              cols 128..512); the host reduces the 128 partitions.
    """
    f32 = mybir.dt.float32
    f16 = mybir.dt.float16
    fp8 = mybir.dt.float8e4
    ngrp = KCH // KSTEP

    nc = bacc.Bacc("TRN2", target_bir_lowering=False, debug=False,
                   num_devices=N_CORES)
    xt = nc.dram_tensor("xt", [NSB * P, KCH * NBLK], fp8,
                        kind="ExternalInput")
    eyed = nc.dram_tensor("eyef16", [P, P], f16, kind="ExternalInput")
    rowmax_d = nc.dram_tensor("rowmax", [P, NA * 4], f32,
                              kind="ExternalOutput")
    colmax_d = nc.dram_tensor("colmax", [N_SLOTS * P, NBLK], f16,
                              kind="ExternalOutput")
    xt_ap = xt[:]
    colmax_ap = colmax_d[:]

    with tile.TileContext(nc) as tc:
        with (
            tc.tile_pool(name="lhs", bufs=1) as lhs_pool,
            tc.tile_pool(name="rhs", bufs=4) as rhs_pool,
            tc.tile_pool(name="psum", bufs=4, space="PSUM") as psum_pool,
            tc.tile_pool(name="stage", bufs=5) as stage_pool,
            tc.tile_pool(name="stats", bufs=1) as stats_pool,
        ):
            # --- input DMAs, all on the Sync queue. Order matters: lhs0
            # first (the first super-block needs only it), then the eye
            # (not consumed until the first diagonal epilogue); lhs1..3
            # are issued from inside the loop so early rhs prefetches
            # aren't stuck behind them.
            eye = stats_pool.tile([P, P], f16, name="eye")

            lhs_tiles = [
                lhs_pool.tile([P, KCH, NBLK], fp8, name=f"lhs{a}",
                              tag=f"lhs{a}")
                for a in range(NA)
            ]

            def load_lhs(a):
                rows = slice(8 * a * P, (8 * a + 1) * P)
                nc.sync.dma_start(lhs_tiles[a][:], xt_ap[rows, :])

            load_lhs(0)
            nc.sync.dma_start(eye[:], eyed[:])

            racc = [
                stats_pool.tile([P, 4, NBLK], f16, name=f"racc{a}",
                                tag=f"racc{a}")
                for a in range(NA)
            ]
            rowmax_sb = stats_pool.tile([P, NA * 4], f32, name="rowmax_sb")
            rowtmp = stats_pool.tile([P, 4], f32, name="rowtmp")
            # scratch for the last block's j-fold tree
            stj = stats_pool.tile([P, 4, NBLK // 2], f16, name="stj")
            stj2 = stats_pool.tile([P, 4, NBLK // 4], f16, name="stj2")

            # --- PE warm-up: tiny matmuls on a never-written SBUF
            # scratch tile (garbage data, zero input dependencies — they
            # start the moment the Tensor queue is up, before any DMA
            # lands) fill the HAM activity window while lhs0 streams in.
            # They write into the first block's pre-allocated psum half;
            # its first real matmul's start=True clears has_written. ---
            wsc = stats_pool.tile([P, P], f16, name="wsc")
            nc.vector.memset(wsc[:], 0.25)
            first_ps = psum_pool.tile([P, 2, NBLK], f32, name="ps",
                                      tag="ps")
            for _ in range(N_WARMUP):
                nc.tensor.matmul(first_ps[:, 0, 0:P], wsc[:], wsc[:],
                                 start=True, stop=True)

            def row_reduce(dst_ap, src_tile):
                """dst[:, 0:4] = per-chunk row max of src [P, 4, NBLK]."""
                nc.vector.reduce_max(
                    out=dst_ap,
                    in_=src_tile[:],
                    axis=mybir.AxisListType.X,
                    op=mybir.AluOpType.max,
                )

            sb_idx = 0
            # colmax DMA issues ride the Scalar queue, deferred by one
            # block: the issue for block w waits on its DVE folds, so
            # emitting it after block w+1's ACT copies keeps the PSUM
            # drain path unblocked (Vector cannot issue DMAs; GpSimd
            # DMA rings pay a ~6.5us software drain at teardown; Sync
            # issues would head-of-line-block the rhs prefetches).
            pending_dma = []

            def flush_colmax():
                while pending_dma:
                    dst, src_ = pending_dma.pop(0)
                    nc.scalar.dma_start(dst, src_)

            for a, w in _windows():
                L = (8 * a + w) % NSB
                wmax = 16 if a < 2 else 15
                if L % 8 == 0:
                    rt = lhs_tiles[L // 8]  # resident (w = 0, 8, 16)
                else:
                    rt = rhs_pool.tile([P, KCH, NBLK], fp8, name="rt",
                                       tag="rt")
                    nc.sync.dma_start(rt[:], xt_ap[L * P:(L + 1) * P, :])
                if a == 0 and 1 <= w <= 3:
                    load_lhs(w)  # deferred so early rhs loads go first

                # two 2-bank psum halves per block: ACT staging starts
                # after 8 matmuls instead of 16 and the PE can run two
                # blocks ahead of the epilogue (4 pool bufs = 8 banks)
                if first_ps is not None:
                    psh0, first_ps = first_ps, None
                else:
                    psh0 = psum_pool.tile([P, 2, NBLK], f32, name="ps",
                                          tag="ps")
                st = stage_pool.tile([P, 4, NBLK], f16, name="st", tag="st")
                if w == 0:
                    # diagonal: chunk r covers block cols 128r..512 only
                    psh1 = None
                    for r in range(4):
                        if r == 2:
                            psh1 = psum_pool.tile([P, 2, NBLK], f32,
                                                  name="ps", tag="ps")
                        ph = psh0 if r < 2 else psh1
                        nw = NBLK - r * P
                        for g in range(ngrp):
                            ks = slice(KSTEP * g, KSTEP * (g + 1))
                            nc.tensor.matmul(
                                ph[:, r % 2, 0:nw],
                                lhs_tiles[a][:, ks, r * P:(r + 1) * P],
                                lhs_tiles[a][:, ks, r * P:NBLK],
                                start=(g == 0),
                                stop=(g == ngrp - 1),
                                perf_mode=mybir.MatmulPerfMode.DoubleRow,
                            )
                        if r == 1:  # stage half 0 while half 1 computes
                            nc.scalar.copy(st[:, 0, :], psh0[:, 0, :])
                            nc.scalar.copy(st[:, 1, 0:NBLK - P],
                                           psh0[:, 1, 0:NBLK - P])
                    nc.scalar.copy(st[:, 2, 0:NBLK - 2 * P],
                                   psh1[:, 0, 0:NBLK - 2 * P])
                    nc.scalar.copy(st[:, 3, 0:NBLK - 3 * P],
                                   psh1[:, 1, 0:NBLK - 3 * P])
                    flush_colmax()
                    # self-sim sits at st[p, r, p]: mask it below any
                    # real similarity before any max consumes it
                    for r in range(4):
                        sl = st[:, r, 0:P]
                        nc.vector.tensor_add(out=sl, in0=sl, in1=eye[:])
                    # init racc: ragged copy + -inf tails
                    nc.vector.tensor_copy(racc[a][:, 0, :], st[:, 0, :])
                    for r in range(1, 4):
                        nw = NBLK - r * P
                        nc.vector.tensor_copy(racc[a][:, r, 0:nw],
                                              st[:, r, 0:nw])
                        nc.vector.memset(racc[a][:, r, nw:NBLK], -60000.0)
                    # ragged column-max over the computed rows, block
                    # cols 128..512 (col offset 128r+j in chunk r maps to
                    # local j after the per-chunk 128-col shift), folded
                    # in place into st chunk 0 (racc copies ran already)
                    nc.vector.tensor_max(out=st[:, 0, P:NBLK],
                                         in0=st[:, 0, P:NBLK],
                                         in1=st[:, 1, 0:DIAG_W])
                    nc.vector.tensor_max(out=st[:, 0, 2 * P:NBLK],
                                         in0=st[:, 0, 2 * P:NBLK],
                                         in1=st[:, 2, 0:NBLK - 2 * P])
                    nc.vector.tensor_max(out=st[:, 0, 3 * P:NBLK],
                                         in0=st[:, 0, 3 * P:NBLK],
                                         in1=st[:, 3, 0:P])
                    pending_dma.append(
                        (colmax_ap[sb_idx * P:(sb_idx + 1) * P, 0:DIAG_W],
                         st[:, 0, P:NBLK]))
                else:
                    last = (a == NA - 1 and w == wmax)
                    psh1 = None
                    for r in range(4):
                        if r == 2:
                            psh1 = psum_pool.tile([P, 2, NBLK], f32,
                                                  name="ps", tag="ps")
                        ph = psh0 if r < 2 else psh1
                        for g in range(ngrp):
                            ks = slice(KSTEP * g, KSTEP * (g + 1))
                            nc.tensor.matmul(
                                ph[:, r % 2, :],
                                lhs_tiles[a][:, ks, r * P:(r + 1) * P],
                                rt[:, ks, :],
                                start=(g == 0),
                                stop=(g == ngrp - 1),
                                perf_mode=mybir.MatmulPerfMode.DoubleRow,
                            )
                        if r == 1:  # stage half 0 while half 1 computes
                            nc.scalar.copy(st[:, 0:2, :], psh0[:])
                        elif r == 2 and last:
                            # per-chunk staging: chunk 2 copies while
                            # chunk 3's matmuls run, shortening the tail
                            nc.scalar.copy(st[:, 2, :], psh1[:, 0, :])
                    if last:
                        nc.scalar.copy(st[:, 3, :], psh1[:, 1, :])
                    else:
                        nc.scalar.copy(st[:, 2:4, :], psh1[:])
                    flush_colmax()
                    # the last two blocks bypass racc (their row maxes go
                    # through the fold tree below) so the only DVE work
                    # left after the final matmul is one block's epilogue
                    direct = (a == NA - 1 and w >= wmax - 1)

                    if not direct:
                        # running row-max (f16 2x tensor_tensor)
                        nc.vector.tensor_max(out=racc[a][:],
                                             in0=racc[a][:], in1=st[:])
                    else:
                        # last two blocks bypass racc: j-fold st (must
                        # read it before the in-place colmax folds below)
                        nc.vector.tensor_max(out=stj[:],
                                             in0=st[:, :, 0:NBLK // 2],
                                             in1=st[:, :, NBLK // 2:NBLK])
                    # column-max fold to [128, 512], in place into st
                    # chunks 0-1 then 0 (racc/fold-tree consumed st
                    # beforehand); host folds the 128 partitions
                    nc.vector.tensor_max(out=st[:, 0:2, :],
                                         in0=st[:, 0:2, :],
                                         in1=st[:, 2:4, :])
                    nc.vector.tensor_max(out=st[:, 0, :],
                                         in0=st[:, 0, :],
                                         in1=st[:, 1, :])
                    pending_dma.append(
                        (colmax_ap[sb_idx * P:(sb_idx + 1) * P, :],
                         st[:, 0, :]))
                    if direct:
                        # small reduce + merge (racc for this window was
                        # reduced after w = wmax-2, two blocks of matmul
                        # cover before the kernel ends)
                        nc.vector.tensor_max(out=stj2[:],
                                             in0=stj[:, :, 0:NBLK // 4],
                                             in1=stj[:, :, NBLK // 4:NBLK // 2])
                        row_reduce(rowtmp[:], stj2)
                        nc.vector.tensor_max(
                            out=rowmax_sb[:, 4 * a:4 * a + 4],
                            in0=rowmax_sb[:, 4 * a:4 * a + 4],
                            in1=rowtmp[:])
                sb_idx += 1

                # per-window row-max reduction (hidden under the next
                # block's matmuls; for the very last window it runs two
                # blocks early — the DVE lags the PE by about a block, so
                # anything issued later still lands after the final MM)
                red_now = (w == wmax - 2) if (a == NA - 1) else (w == wmax)
                if red_now:
                    row_reduce(rowmax_sb[:, 4 * a:4 * a + 4], racc[a])

            flush_colmax()
            nc.sync.dma_start(rowmax_d[:], rowmax_sb[:])

    nc.compile()
    return nc


def prepare_inputs(x):
    """Host prep: normalize (f64), scale+cast fp8, retile, rotate."""
    xd = np.asarray(x, dtype=np.float64)
    norms = np.sqrt(np.einsum("ij,ij->i", xd, xd))
    np.maximum(norms, 1e-12, out=norms)
    xn = xd / norms[:, None]
    xnt = (xn.T * FP8_SCALE).astype(ml_dtypes.float8_e4m3)  # [D, B]
    # retile to [J, p, k, j]: xt_r[J, p, k, j] = xnt[k*128+p, J*512+j]
    xt_r = np.ascontiguousarray(
        xnt.reshape(KCH, P, NSB, NBLK).transpose(2, 1, 0, 3))
    eyef16 = np.ascontiguousarray(
        EYE_VAL * np.eye(P, dtype=np.float32)).astype(np.float16)
    in_maps = []
    for c in range(N_CORES):
        rot = (np.concatenate([xt_r[c:], xt_r[:c]], axis=0) if c
               else xt_r)
        in_maps.append({
            "xt": np.ascontiguousarray(rot).reshape(NSB * P, KCH * NBLK),
            "eyef16": eyef16,
        })
    return in_maps


def postprocess(results):
    """Merge per-core row/column maxima and apply the scalar epilogue."""
    inv = 1.0 / (FP8_SCALE * FP8_SCALE)
    order = _windows()
    maxsim = np.full(B, -np.inf, dtype=np.float64)
    for c in range(N_CORES):
        rm = np.asarray(results[c]["rowmax"], dtype=np.float64)  # [P, 16]
        for a in range(NA):
            for r in range(4):
                g0 = (c + 8 * a) * NBLK + r * P  # global row of partition 0
                sl = slice(g0, g0 + P)
                np.maximum(maxsim[sl], rm[:, 4 * a + r], out=maxsim[sl])
        cmx = np.asarray(results[c]["colmax"]).astype(np.float32)
        cmx = cmx.reshape(N_SLOTS, P, NBLK).max(axis=1).astype(np.float64)
        for s, (a, w) in enumerate(order):
            g0 = ((8 * a + w + c) % NSB) * NBLK
            if w == 0:
                # diag slot: cols 0:384 cover block cols 128..512
                sl = slice(g0 + P, g0 + NBLK)
                np.maximum(maxsim[sl], cmx[s, 0:DIAG_W], out=maxsim[sl])
            else:
                sl = slice(g0, g0 + NBLK)
                np.maximum(maxsim[sl], cmx[s], out=maxsim[sl])
    d2 = 2.0 - 2.0 * (maxsim * inv) + EPS
    loss = -0.5 * np.mean(np.log(d2))
    return np.array(loss, dtype=np.float32)


_NC_CACHE = {}


def _get_nc():
    if "nc" not in _NC_CACHE:
        _NC_CACHE["nc"] = build_nc()
    return _NC_CACHE["nc"]


def kernel(x, **_ignored):
    import time

    nc = _get_nc()
    in_maps = prepare_inputs(x)
    last_exc = None
    for attempt in range(3):
        try:
            res = run_bass_kernel_spmd(nc, in_maps,
                                       core_ids=list(range(N_CORES)))
            return postprocess(res.results)
        except Exception as exc:  # transient NRT/tunnel hiccups
            last_exc = exc
            if attempt < 2:
                time.sleep(30)  # a wedged exec unit takes a while to heal
    raise last_exc


if __name__ == "__main__":
    x = np.random.default_rng(0).standard_normal((B, D), dtype=np.float32)
    print(kernel(x))
